# revision 19
# baseline (speedup 1.0000x reference)
"""ColumnParallelLinearWithDelta: GPTQ-int4 LoRA-delta matmul on 8 trn2 cores.

out[d] = x @ dequant(qweight[d], qzeros[d], scales[d]) + x @ base_weight.T

Sharding: column-parallel — out_features (4096) split into 8 slices of 512,
one per NeuronCore; x replicated. Each core computes its [8, 256, 512] slice
of the delta stack plus the shared base output; the host adds base during
the unshard (exact f32 broadcast-add, O(output) work).

Math (per core, out-col slice ns):
  W[k, n]  = s[g(k), n] * (w4[k, n] - (z4[g(k), n] + 1)),  g(k) = k // 128
  delta    = x @ W = x @ (s .* w4)  -  xs @ (s .* (z4 + 1))
  with xs[t, g] = sum_{k in g} x[t, k]   (host-precomputed group sums)
  out[d]   = delta_d + base,  base = x @ base_weight[ns, :].T

Layout trick (all host-side shuffles): each packed int32 row r holds nibbles
j = 0..7 of input rows k = 8r+j; viewing it as int16, the even halfwords
hold j = 0..3 and the odd halfwords j = 4..7 of the same output column n.
The host splits each row's even/odd halfwords across two partition halves
(p = 64*e + r64, 64-row chunks c), so ONE tensor_scalar (>> 4sh) & 0xF over
a [128, F] slice yields, per shift sh, a DENSE [128, 512]-per-chunk nibble
plane: partitions p < 64 carry j = sh, partitions p >= 64 carry j = 4+sh,
free dim = n. 16-bit DVE ops run in 2x mode (int8 would halve the rate).
Each plane is multiplied by a partition-replicated scale tile and consumed
as the matmul moving operand against a host-reordered stationary x tile —
contraction rows k(p) = 8*(64c + p%64) + 4*(p//64) + sh.

Schedule (the perf-critical part; idle PE re-engages the HAM clock gate so
gaps cost double):
 - Startup: the host pre-dequantizes adapter 0's first two chunks (d_v0,
   1 MB) so the first ~7us of matmuls have no DVE dependency; DMAs are
   enqueued in priority order (xt chunk 0, v0 plane 0, ...) and the first
   weight quarter feeds the dequant pipeline while v0 matmuls run.
 - Dequant is sliced in half/quarter planes so each TT releases matmuls
   early; TS emits fp16 so the TT multiply runs fp16 x fp16.
 - The base matmul is split into per-chunk groups placed between adapters
   as PE shock absorbers; its wb stream trickles through the weight queue
   in FIFO order so it never starves the adapter weight stream.
 - Tail: the last adapter drains PSUM in halves on both ACT and DVE with
   output DMAs on the (idle) sync queue.
"""

import numpy as np

# ---- problem constants (hardcoded; kernel.py must be self-contained) ----
T = 256          # tokens
IN = 4096        # in_features
OUT = 4096       # out_features
D = 8            # adapters
GROUP = 128      # quant group size
G = IN // GROUP  # 32 groups
NCORES = 8
NC_OUT = OUT // NCORES   # 512 out cols per core
NCH = 8                  # contraction chunks of 64 packed rows (x2 e-halves)
FD = NCH * NC_OUT        # 4096
WARM = 16                # PE clock-warm matmuls (N=256 each): the HAM clock
                         # gate needs ~4.7us of CONTINUOUS PE activity to
                         # lift, and any idle gap resets the ramp — the warm
                         # stream must bridge until the first data arrives

_PROGRAM_CACHE: dict = {}


def _build_program():
    import concourse.bacc as bacc
    import concourse.mybir as mybir
    import concourse.tile as tile

    nc = bacc.Bacc("TRN2", target_bir_lowering=False, debug=False)

    fp16 = mybir.dt.float16
    i16 = mybir.dt.int16
    f32 = mybir.dt.float32

    d_xt = nc.dram_tensor("xt", (128, NCH * 4 * T), fp16, kind="ExternalInput")
    d_negxs = nc.dram_tensor("negxs", (G, T), fp16, kind="ExternalInput")
    d_qw16 = nc.dram_tensor("qw16", (D, 2, 128, FD // 2), i16,
                            kind="ExternalInput")
    d_s2 = nc.dram_tensor("s2", (D, 2, 128, FD // 2), fp16,
                          kind="ExternalInput")
    d_v0 = nc.dram_tensor("v0", (128, 4 * 2 * NC_OUT), fp16,
                          kind="ExternalInput")
    d_wb = nc.dram_tensor("wb", (NCH, 128, 4 * NC_OUT), fp16,
                          kind="ExternalInput")
    d_sz = nc.dram_tensor("sz", (G, D * NC_OUT), fp16, kind="ExternalInput")
    d_out = nc.dram_tensor("out", (D, T, NC_OUT), f32, kind="ExternalOutput")
    d_outb = nc.dram_tensor("outb", (T, NC_OUT), f32, kind="ExternalOutput")

    AT = mybir.AluOpType

    with tile.TileContext(nc) as tc:
        with (
            tc.tile_pool(name="const", bufs=1) as cpool,
            tc.tile_pool(name="qw", bufs=2) as qpool,
            tc.tile_pool(name="s2", bufs=2) as spool,
            tc.tile_pool(name="vr", bufs=1) as vrpool,
            tc.tile_pool(name="v", bufs=2) as vpool,
            tc.tile_pool(name="wb", bufs=2) as wpool,
            tc.tile_pool(name="outp", bufs=4) as opool,
            tc.tile_pool(name="ps", bufs=2, space="PSUM") as ppool,
            tc.tile_pool(name="psb", bufs=1, space="PSUM") as pbpool,
        ):
            xt_sb = cpool.tile([128, NCH * 4 * T], fp16)
            negxs_sb = cpool.tile([G, T], fp16)
            sz_sb = cpool.tile([G, D * NC_OUT], fp16)
            warm_sb = cpool.tile([128, 384], fp16)
            v0_sb = cpool.tile([128, 4 * 2 * NC_OUT], fp16)

            first_ts = [None]

            def xt_tile(c, sh, th):
                off = (c * 4 + sh) * T + th * 128
                return xt_sb[:, off:off + 128]

            def xt_dma(c):
                off = c * 4 * T
                return nc.scalar.dma_start(xt_sb[:, off:off + 4 * T],
                                           d_xt[:, off:off + 4 * T])

            def ts_tt(qw_t, s2_t, sh, f0, f1, gp=False):
                """Nibble plane sh over free cols [f0:f1]: TS (shift+and on
                int16 — bitwise ops cannot cast) then TT scale multiply.
                gp=True runs the TT on GpSimd to relieve the DVE."""
                vr = vrpool.tile([128, FD], i16, tag=f"vr{sh}",
                                 name=f"vr{sh}")
                ts_i = nc.vector.tensor_scalar(
                    out=vr[:, f0:f1], in0=qw_t[:, f0:f1],
                    scalar1=4 * sh, scalar2=0xF,
                    op0=AT.logical_shift_right, op1=AT.bitwise_and,
                )
                if first_ts[0] is None:
                    first_ts[0] = ts_i
                v = vpool.tile([128, FD], fp16, tag=f"v{sh}", name=f"v{sh}")
                eng = nc.gpsimd if gp else nc.vector
                eng.tensor_tensor(
                    out=v[:, f0:f1], in0=vr[:, f0:f1], in1=s2_t[:, f0:f1],
                    op=AT.mult)
                return v

            def plane_matmuls(ps, v, sh, c0, c1, first):
                for c in range(c0, c1):
                    rhs = v[:, c * NC_OUT:(c + 1) * NC_OUT]
                    for th in range(2):
                        nc.tensor.matmul(
                            ps[th][:], lhsT=xt_tile(c, sh, th), rhs=rhs,
                            start=(first and c == c0), stop=False)

            def adapter_finish(d, ps, split_tail=False):
                """Zeros correction; drain PSUM (base added on host)."""
                for th in range(2):
                    nc.tensor.matmul(
                        ps[th][:],
                        lhsT=negxs_sb[:, th * 128:(th + 1) * 128],
                        rhs=sz_sb[:, d * NC_OUT:(d + 1) * NC_OUT],
                        start=False, stop=True,
                    )
                if not split_tail:
                    for th in range(2):
                        o_t = opool.tile([128, NC_OUT], f32, name="o_t")
                        nc.scalar.copy(o_t[:], ps[th][:])
                        nc.scalar.dma_start(
                            d_out[d, th * 128:(th + 1) * 128, :], o_t[:])
                else:
                    # final adapter: drain in halves on both ACT and DVE so
                    # the last output DMAs start early; DMAs on the idle
                    # sync queue
                    for th in range(2):
                        o_t = opool.tile([128, NC_OUT], f32, name="o_t")
                        h = NC_OUT // 2
                        nc.scalar.copy(o_t[:, :h], ps[th][:, :h])
                        nc.vector.tensor_scalar_add(o_t[:, h:], ps[th][:, h:],
                                                    0.0)
                        nc.sync.dma_start(
                            d_out[d, th * 128:(th + 1) * 128, :h], o_t[:, :h])
                        nc.sync.dma_start(
                            d_out[d, th * 128:(th + 1) * 128, h:], o_t[:, h:])

            # ---------------- startup: priority DMA interleave ----------------
            # strict need-order: the PE consumes v0 plane sh at ~0.9us each
            # from ~11us while the dequant pipeline needs qw/s2 quarters in
            # parallel — interleave so nothing waits on bytes it doesn't need
            nc.vector.memset(warm_sb[:], 0.0)
            xt_dma(0)
            qw_t0 = qpool.tile([128, FD], i16, tag="qw", name="qw_t")
            s2_t0 = spool.tile([128, FD], fp16, tag="s2", name="s2_t")

            def v0_dma(sh):
                nc.sync.dma_start(v0_sb[:, sh * 1024:(sh + 1) * 1024],
                                  d_v0[:, sh * 1024:(sh + 1) * 1024])

            def qs_qtr(q):
                # adapter-0 qw/s2 quarter q (chunks 2q..2q+1); quarter 0 is
                # pre-dequantized via v0 and never loaded
                h, r = divmod(q, 2)
                nc.sync.dma_start(
                    qw_t0[:, q * 1024:(q + 1) * 1024],
                    d_qw16[0, h, :, r * 1024:(r + 1) * 1024])
                nc.sync.dma_start(
                    s2_t0[:, q * 1024:(q + 1) * 1024],
                    d_s2[0, h, :, r * 1024:(r + 1) * 1024])

            v0_dma(0)
            qs_qtr(1)
            xt_dma(1)
            v0_dma(1)
            xt_dma(2)
            qs_qtr(2)
            v0_dma(2)
            xt_dma(3)
            v0_dma(3)
            qs_qtr(3)
            # non-urgent startup transfers: gated behind the first dequant
            # op so the priority stream above gets all the bandwidth
            late_dmas = [xt_dma(c) for c in range(4, NCH)]
            late_dmas.append(nc.scalar.dma_start(negxs_sb[:], d_negxs[:]))
            late_dmas.append(nc.scalar.dma_start(sz_sb[:], d_sz[:]))

            with tc.tile_pool(name="warmps", bufs=1, space="PSUM") as wpsp:
                warm_ps = wpsp.tile([128, 256], f32)
                for _ in range(WARM):
                    nc.tensor.matmul(
                        warm_ps[:], lhsT=warm_sb[:, :128],
                        rhs=warm_sb[:, 128:384], start=True, stop=True)

            ps0 = [ppool.tile([128, NC_OUT], f32, tag=f"ps{t}", name=f"ps{t}")
                   for t in range(2)]
            # chunks 0-1 from host-dequantized v0 (no DVE dependency)
            for sh in range(4):
                for c in range(2):
                    rhs = v0_sb[:, sh * 1024 + c * NC_OUT:
                                sh * 1024 + (c + 1) * NC_OUT]
                    for th in range(2):
                        nc.tensor.matmul(
                            ps0[th][:], lhsT=xt_tile(c, sh, th), rhs=rhs,
                            start=(sh == 0 and c == 0), stop=False)

            # quarter-granular dequant for chunks 2-7 (quarters q=1..3),
            # sh=3 multiplies on GpSimd
            for q in range(1, 4):
                for sh in range(4):
                    v = ts_tt(qw_t0, s2_t0, sh, q * 1024, (q + 1) * 1024,
                              gp=(sh == 3))
                    plane_matmuls(ps0, v, sh, 2 * q, 2 * q + 2, False)
                if q == 1:
                    for dma in late_dmas:
                        tile.add_dep_helper(
                            dma.ins, first_ts[0].ins,
                            reason="gate non-urgent startup DMAs")
            adapter_finish(0, ps0)

            # ---------------- steady state ----------------
            ps_b = [pbpool.tile([128, NC_OUT], f32, tag=f"psb{t}",
                                name=f"psb{t}") for t in range(2)]

            def base_chunk(c):
                wb_t = wpool.tile([128, 4 * NC_OUT], fp16, name="wb_t",
                                  tag="wb")
                nc.sync.dma_start(wb_t[:], d_wb[c, :, :])
                for sh in range(4):
                    rhs = wb_t[:, sh * NC_OUT:(sh + 1) * NC_OUT]
                    for th in range(2):
                        nc.tensor.matmul(
                            ps_b[th][:], lhsT=xt_tile(c, sh, th), rhs=rhs,
                            start=(c == 0 and sh == 0),
                            stop=(c == NCH - 1 and sh == 3))

            def adapter_main(d, ps, halves):
                """halves=True slices dequant ops in two (lower latency,
                ~2us more DVE time per adapter) — used while the pipeline
                lead is still thin; later adapters use full planes."""
                qw_t = qpool.tile([128, FD], i16, tag="qw", name="qw_t")
                s2_t = spool.tile([128, FD], fp16, tag="s2", name="s2_t")
                for h in range(2):
                    nc.sync.dma_start(qw_t[:, h * 2048:(h + 1) * 2048],
                                      d_qw16[d, h, :, :])
                    nc.sync.dma_start(s2_t[:, h * 2048:(h + 1) * 2048],
                                      d_s2[d, h, :, :])
                if halves:
                    for h in range(2):
                        for sh in range(4):
                            v = ts_tt(qw_t, s2_t, sh, h * 2048,
                                      (h + 1) * 2048, gp=(sh == 3))
                            plane_matmuls(ps, v, sh, h * 4, (h + 1) * 4,
                                          first=(h == 0 and sh == 0))
                else:
                    for sh in range(4):
                        v = ts_tt(qw_t, s2_t, sh, 0, FD, gp=(sh == 3))
                        plane_matmuls(ps, v, sh, 0, NCH, first=(sh == 0))

            def adapter_tail(d, ps):
                """Last adapter: run token-half th=0 to completion and drain
                it while th=1's matmuls still stream — the final output DMA
                is the only thing left after the last matmul."""
                qw_t = qpool.tile([128, FD], i16, tag="qw", name="qw_t")
                s2_t = spool.tile([128, FD], fp16, tag="s2", name="s2_t")
                for h in range(2):
                    nc.sync.dma_start(qw_t[:, h * 2048:(h + 1) * 2048],
                                      d_qw16[d, h, :, :])
                    nc.sync.dma_start(s2_t[:, h * 2048:(h + 1) * 2048],
                                      d_s2[d, h, :, :])
                planes = {}
                for sh in range(4):
                    planes[sh] = ts_tt(qw_t, s2_t, sh, 0, FD,
                                       gp=(sh in (1, 3)))
                for th in range(2):
                    for sh in range(4):
                        v = planes[sh]
                        for c in range(NCH):
                            rhs = v[:, c * NC_OUT:(c + 1) * NC_OUT]
                            nc.tensor.matmul(
                                ps[th][:], lhsT=xt_tile(c, sh, th), rhs=rhs,
                                start=(sh == 0 and c == 0), stop=False)
                    nc.tensor.matmul(
                        ps[th][:],
                        lhsT=negxs_sb[:, th * 128:(th + 1) * 128],
                        rhs=sz_sb[:, d * NC_OUT:(d + 1) * NC_OUT],
                        start=False, stop=True,
                    )
                    o_t = opool.tile([128, NC_OUT], f32, name="o_t")
                    if th == 0:
                        nc.scalar.copy(o_t[:], ps[th][:])
                    else:
                        nc.vector.tensor_scalar_add(o_t[:], ps[th][:], 0.0)
                    nc.sync.dma_start(
                        d_out[d, th * 128:(th + 1) * 128, :], o_t[:])

            # base chunk slots: c0 after a0; c1,c2 after a1; c3,c4 after a2;
            # c5..c7 after a3..a5; outb drains after a6 (during a7's window)
            base_chunk(0)
            base_slots = {1: [1, 2], 2: [3, 4], 3: [5], 4: [6], 5: [7]}
            for d in range(1, D):
                ps = [ppool.tile([128, NC_OUT], f32, tag=f"ps{t}",
                                 name=f"ps{t}") for t in range(2)]
                if d == D - 1:
                    adapter_tail(d, ps)
                else:
                    adapter_main(d, ps, halves=(d == 1))
                    adapter_finish(d, ps)
                for c in base_slots.get(d, []):
                    base_chunk(c)
                if d == 6:
                    for th in range(2):
                        ob_t = opool.tile([128, NC_OUT], f32, name="ob_t")
                        nc.scalar.copy(ob_t[:], ps_b[th][:])
                        nc.scalar.dma_start(
                            d_outb[th * 128:(th + 1) * 128, :], ob_t[:])

    nc.compile()
    return nc


def _prep_inputs(x, base_weight, qweight, qzeros, scales):
    """Host-side layout prep. Returns list of 8 per-core input maps."""
    x = np.asarray(x, dtype=np.float32)
    base_weight = np.asarray(base_weight, dtype=np.float32)
    qweight = np.asarray(qweight, dtype=np.int32)
    qzeros = np.asarray(qzeros, dtype=np.int32)
    scales = np.asarray(scales, dtype=np.float32)

    # stationary x tiles: xt[64e + r64, (4c+sh)*T + t] = x[t, 8*(64c+r64)
    # + 4e + sh]  — matches the dense nibble-plane partition layout
    xr = np.ascontiguousarray(x.T).reshape(NCH, 64, 2, 4, T)  # [c,r64,e,sh,t]
    xt = np.ascontiguousarray(xr.transpose(2, 1, 0, 3, 4))    # [e,r64,c,sh,t]
    xt = xt.reshape(128, NCH * 4 * T).astype(np.float16)

    # group sums of x (for the zeros-correction contraction), negated
    xs = x.reshape(T, G, GROUP).sum(axis=2)                   # [t, g]
    negxs = np.ascontiguousarray((-xs.T)).astype(np.float16)  # [g, t]

    # unpack qzeros (packed along out cols): z4[d, g, 8m+jj]
    jj = 4 * np.arange(8, dtype=np.int32)
    z4 = ((qzeros[:, :, :, None] >> jj[None, None, None, :]) & 0xF)
    z4 = z4.reshape(D, G, OUT)                                # [d, g, n]
    sz_full = scales * (z4 + 1).astype(np.float32)            # [d, g, n]

    # adapter-0 chunks 0-1 pre-dequantized: w4 rows k in [0, 1024)
    w4_01 = ((qweight[0][:128, None, :] >> jj[None, :, None]) & 0xF)
    w4_01 = w4_01.reshape(1024, OUT).astype(np.float32)       # [k, n]
    g01 = np.arange(1024) // GROUP
    v0_full = scales[0][g01] * w4_01                          # [1024, n]

    in_maps = []
    for core in range(NCORES):
        ns = slice(core * NC_OUT, (core + 1) * NC_OUT)

        # packed weights: partition p = 64e + r64 holds the e-half words of
        # packed row 64c + r64; free = chunk-major, n dense within chunk
        qw_c = np.ascontiguousarray(qweight[:, :, ns])        # [D, 512, 512]
        qw16 = qw_c.view(np.int16).reshape(D, NCH, 64, NC_OUT, 2)
        qw16 = np.ascontiguousarray(qw16.transpose(0, 4, 2, 1, 3))
        # [d, e, r64, c, n] -> [D, 2 halves, 128, FD/2]
        qw16 = qw16.reshape(D, 128, NCH * NC_OUT)
        qw16 = qw16.reshape(D, 128, 2, FD // 2).transpose(0, 2, 1, 3)

        # scale tile: s2[d, p, c*512+n] = s[d, 4c + (p%64)//16, n]
        s_c = scales[:, :, ns]                                # [D, G, 512]
        s2 = s_c.reshape(D, NCH, 4, NC_OUT)                   # [d, c, g4, n]
        s2 = np.repeat(s2, 16, axis=2)                        # [d, c, 64, n]
        s2 = np.broadcast_to(s2[:, None], (D, 2, NCH, 64, NC_OUT))
        s2 = np.ascontiguousarray(s2.transpose(0, 1, 3, 2, 4))  # [d,e,r64,c,n]
        s2 = s2.reshape(D, 128, NCH * NC_OUT).astype(np.float16)
        s2 = s2.reshape(D, 128, 2, FD // 2).transpose(0, 2, 1, 3)

        # v0: adapter-0 chunks 0-1 planes, v0[64e+r64, sh*1024 + c*512 + n]
        # = s*w4 at k = 512c + 8*r64 + 4e + sh
        v0c = v0_full[:, ns]                                  # [1024, 512]
        v0 = v0c.reshape(2, 64, 2, 4, NC_OUT)                 # [c,r64,e,sh,n]
        v0 = np.ascontiguousarray(v0.transpose(2, 1, 3, 0, 4))  # [e,r64,sh,c,n]
        v0 = v0.reshape(128, 4 * 2 * NC_OUT).astype(np.float16)

        # base weights in the same sub-chunk order: wb[c][p, sh*512+n] =
        # base_weight[ns, :].T[k(p, c, sh), n]
        bw_c = base_weight[ns, :]                             # [512, 4096]
        wb = np.ascontiguousarray(bw_c.T).reshape(NCH, 64, 2, 4, NC_OUT)
        wb = np.ascontiguousarray(wb.transpose(0, 2, 1, 3, 4))  # [c,e,r64,sh,n]
        wb = wb.reshape(NCH, 128, 4 * NC_OUT).astype(np.float16)

        sz_c = sz_full[:, :, ns]                              # [D, G, 512]
        sz = np.ascontiguousarray(sz_c.transpose(1, 0, 2)).reshape(G, D * NC_OUT)
        sz = sz.astype(np.float16)

        in_maps.append({
            "xt": xt, "negxs": negxs,
            "qw16": np.ascontiguousarray(qw16),
            "s2": np.ascontiguousarray(s2),
            "v0": v0,
            "wb": np.ascontiguousarray(wb),
            "sz": sz,
        })
    return in_maps


def _run(in_maps, trace=False):
    from concourse import bass_utils
    if "nc" not in _PROGRAM_CACHE:
        _PROGRAM_CACHE["nc"] = _build_program()
    nc = _PROGRAM_CACHE["nc"]
    res = bass_utils.run_bass_kernel_spmd(
        nc, in_maps, core_ids=list(range(NCORES)), trace=trace
    )
    return res


def kernel(x, base_weight, qweight, qzeros, scales, g_idx, _trace=False,
           _return_results=False):
    in_maps = _prep_inputs(x, base_weight, qweight, qzeros, scales)
    res = _run(in_maps, trace=_trace)
    out = np.concatenate(
        [res.results[c]["out"] + res.results[c]["outb"][None, :, :]
         for c in range(NCORES)], axis=2)
    if _return_results:
        return out, res
    return out
